# revision 39
# baseline (speedup 1.0000x reference)
"""LCNN conv2d kernel for Trainium2 (8 NeuronCores, batch-sharded).

Math: out[b,o,h,w] = sum_d Wmat[o,d] * conv2d(x, dictionary)[b,d,h,w]
where Wmat is the scatter-add of lookup_coefficients into [O, D].

Device strategy (per core, 2 batches), all matmuls bf16 (PE reaches the
2.4 GHz pstate; f32r holds it near 1.2 GHz):
 - stage 1: conv with the D=100 dictionary in 5 accumulating matmuls per
   output tile. Two shifted copies of x pack two kernel taps into the
   128-partition contraction: XA pairs (kh,0)+(kh,1) via a +1 shift,
   XB pairs (0,2)+(1,2) via a +PW shift; only tap (2,2) runs alone.
 - stage 2: [O=256, D=100] channel-mix matmul on the conv result.
 - output staged and stored as bf16 (halves HBM store traffic); host
   upcasts to f32.
"""
import os
import sys

for _p in ("/opt/trn_rl_repo", "/root/.axon_site/_ro/trn_rl_repo"):
    if os.path.isdir(_p) and _p not in sys.path:
        sys.path.insert(0, _p)

import ml_dtypes
import numpy as np
from contextlib import ExitStack

from concourse import bacc, mybir, tile
from concourse.bass_utils import run_bass_kernel_spmd

# problem shapes (hardcoded per contract)
B, CIN, H, W = 16, 64, 96, 96
D, O = 100, 256
NCORES = 8
BPC = B // NCORES          # batches per core
PH, PW = H + 2, W + 2      # zero-padded spatial
F = BPC * PH * PW          # per-partition x extent
R = 4                      # output rows per matmul tile
NT = H // R                # h-tiles per batch
G = 4                      # h-tiles per output-DMA group
NG = NT // G
N = R * W                  # matmul free size (384)
f32 = mybir.dt.float32
bf16 = mybir.dt.bfloat16

_NC_CACHE = {}


def _build():
    nc = bacc.Bacc(None, target_bir_lowering=False, debug=False)
    # host pre-casts to bf16 so loads use the fast no-cast HWDGE path; x is
    # loaded once (2.4 MB) and the three shifted tap-planes are built with
    # SBUF->SBUF DMA (no HBM traffic).
    xp = nc.declare_dram_parameter("xp", [CIN, F], bf16, isOutput=False)
    # all weights in one buffer: one DMA descriptor (issue instructions
    # cost ~700ns each on the in-order queues)
    WTOT = 5 * D + O
    wc = nc.declare_dram_parameter("wc", [128, WTOT], bf16, isOutput=False)
    # output in staging-buffer order (one flat 2D HWDGE store per group;
    # 3D/strided APs would fall back to the ~155 GB/s software DGE); the
    # host un-shuffles for free.
    out = nc.declare_dram_parameter("out", [BPC * NG, 128, 2 * G * N], bf16,
                                    isOutput=True)

    # h-rows covered by each x sub-tile (two overlapping sub-tiles per
    # batch so every conv tile's 6-row window lives in exactly one).
    SOFF = NT // 2 * R         # second sub-tile starts at h=48
    SH = SOFF + 2              # 50 rows each (= PH - SOFF)

    with tile.TileContext(nc) as tc, ExitStack() as ctx:
        sb = ctx.enter_context(tc.tile_pool(name="sb", bufs=1))
        conv1p = ctx.enter_context(tc.tile_pool(name="conv1p", bufs=3))
        stgp = ctx.enter_context(tc.tile_pool(name="stgp", bufs=2))
        pcp = ctx.enter_context(tc.tile_pool(name="pcp", bufs=2, space="PSUM"))
        pop = ctx.enter_context(tc.tile_pool(name="pop", bufs=2, space="PSUM"))

        wc_s = sb.tile([128, WTOT], bf16)
        wa_s = wc_s[:, 0:3 * D]
        wb_s = wc_s[:, 3 * D:5 * D]
        wm_s = wc_s[0:D, 5 * D:5 * D + O]

        # x planes as 4 sub-tiles each (per batch, overlapping half
        # heights). XA = [x; x shifted +2] pairs taps (kh,0)+(kh,2);
        # XB = [x; x shifted +PW] pairs (0,1)+(1,1); only (2,1) runs
        # alone. All shifts are 4-byte aligned, so the planes are built by
        # fast-path DVE/Act copies from the single HBM x load — the DMA
        # engines (the scarcest resource: ~220ns/packet, ~235 GB/s
        # ceiling shared with the output store) move x only once.
        SL = SH * PW
        XAs = [[None, None] for _ in range(BPC)]
        XBs = [[None, None] for _ in range(BPC)]
        for b in range(BPC):
            for s in range(2):
                XAs[b][s] = sb.tile([128, SL], bf16, name=f"xa_{b}_{s}")
                XBs[b][s] = sb.tile([128, SL], bf16, name=f"xb_{b}_{s}")
                # finite filler for tails the zero-weighted or unread upper
                # rows cover (NaN would poison the PSUM)
                nc.gpsimd.memset(XAs[b][s][64:128, SL - 2:SL], 0.0)
                nc.gpsimd.memset(XBs[b][s][64:128, SL - PW:SL], 0.0)

        def load_x(b, s, eng, lo=0, hi=SL):
            XA = XAs[b][s]
            base = (b * PH + s * SOFF) * PW
            eng.dma_start(XA[0:CIN, lo:hi], xp[:, base + lo:base + hi])

        def make_planes(b, s, lo=0, hi=SL):
            # all on DVE: the Act engine runs these 64-partition copies 3x
            # slower and they'd block the evacuation stream
            XA, XB = XAs[b][s], XBs[b][s]
            nc.vector.tensor_copy(XA[64:128, lo:min(hi, SL - 2)],
                                  XA[0:CIN, lo + 2:min(hi + 2, SL)])
            nc.vector.tensor_copy(XB[0:CIN, lo:hi], XA[0:CIN, lo:hi])
            nc.vector.tensor_copy(XB[64:128, lo:min(hi, SL - PW)],
                                  XA[0:CIN, lo + PW:min(hi + PW, SL)])

        # upfront loads: weights + first sub-tile piece ONLY (the DMA
        # engines fair-share across every pending descriptor, so anything
        # else issued now delays the critical first-tile data). b0s1 and
        # batch 1 are issued from the paced scalar stream inside the loop.
        HSL = SL // 2
        nc.sync.dma_start(wc_s[:], wc[:])
        load_x(0, 0, nc.sync, 0, HSL + PW)
        make_planes(0, 0, 0, HSL)
        load_x(0, 0, nc.sync, HSL + PW, SL)
        make_planes(0, 0, HSL, SL)

        # warm the PE pstate ramp (0.65 -> 2.4 GHz after ~3us of activity)
        # with dummy matmuls on a memset scratch tile while the first x
        # piece is still in flight
        scr = sb.tile([128, N], bf16)
        nc.vector.memset(scr[:], 0.0)
        pwp = ctx.enter_context(tc.tile_pool(name="pwp", bufs=1,
                                             space="PSUM"))
        for _ in range(16):
            pw = pwp.tile([64, N], f32, name="pw")
            nc.tensor.matmul(pw[:], scr[:, 0:64], scr[:], start=True,
                             stop=True)

        def conv_stage(b, g, t):
            h0 = (g * G + t) * R
            s = 1 if h0 >= SOFF else 0
            hl = h0 - s * SOFF
            xav = XAs[b][s].rearrange("p (h w) -> p h w", h=SH, w=PW)
            xbv = XBs[b][s].rearrange("p (h w) -> p h w", h=SH, w=PW)
            pc = pcp.tile([D, N], f32, name="pc")
            # taps (kh,0)+(kh,2) for kh=0,1,2 via the +2 shift
            for kh in range(3):
                nc.tensor.matmul(
                    pc[:], wa_s[:, kh * D:(kh + 1) * D],
                    xav[:, hl + kh:hl + kh + R, 0:W],
                    start=(kh == 0), stop=False)
            # taps (0,1)+(1,1) via the +PW shift
            nc.tensor.matmul(
                pc[:], wb_s[:, 0:D],
                xbv[:, hl:hl + R, 1:1 + W],
                start=False, stop=False)
            # tap (2,1) alone: upper weight rows are zero, upper data rows
            # only need to be finite
            nc.tensor.matmul(
                pc[:], wb_s[:, D:2 * D],
                xbv[:, hl + 2:hl + 2 + R, 1:1 + W],
                start=False, stop=True)
            # PSUM conv evacuation on Act (DVE carries po1 + plane copies)
            c1 = conv1p.tile([D, N], bf16, name="c1")
            nc.scalar.copy(c1[:], pc[:])
            return c1

        stg_of = {}

        def mix_stage(b, g, t, c1):
            if t == 0:
                stg_of[(b, g)] = stgp.tile([128, 2 * G * N], bf16, name="stg")
            stg = stg_of[(b, g)]
            po0 = pop.tile([128, N], f32, name="po0")
            po1 = pop.tile([128, N], f32, name="po1")
            nc.tensor.matmul(po0[:], wm_s[:, 0:128], c1[:],
                             start=True, stop=True)
            nc.tensor.matmul(po1[:], wm_s[:, 128:256], c1[:],
                             start=True, stop=True)
            nc.scalar.copy(stg[:, t * N:(t + 1) * N], po0[:])
            nc.vector.tensor_copy(
                stg[:, G * N + t * N:G * N + (t + 1) * N], po1[:])
            if b == BPC - 1 and g == NG - 1 and t % 2 == 1:
                # last group: store per pair-of-tiles halves so the kernel
                # tail is one small store, not a whole-group one
                h0_, h1_ = (t - 1) * N, (t + 1) * N
                nc.sync.dma_start(out[b * NG + g][:, h0_:h1_],
                                  stg[:, h0_:h1_])
                nc.sync.dma_start(
                    out[b * NG + g][:, G * N + h0_:G * N + h1_],
                    stg[:, G * N + h0_:G * N + h1_])
            elif t == G - 1 and not (b == BPC - 1 and g == NG - 1):
                # one flat 2D store per group -> hardware DGE; issued from
                # sync, which is idle after the initial loads
                nc.sync.dma_start(out[b * NG + g], stg[:])

        # software pipeline: the mix matmuls of tile i-1 are issued after
        # the conv matmuls of tile i, so the PE never stalls on the c1
        # PSUM->SBUF evacuation latency.
        tasks = [(b, g, t)
                 for b in range(BPC) for g in range(NG) for t in range(G)]
        # staged issues, each placed so its inputs are long since ready and
        # it lands well before first use (b0s1 used at task 12, b1s0 at
        # 24, b1s1 at 36)
        load_at = {1: (0, 1), 6: (1, 0), 18: (1, 1)}
        planes_at = {6: (0, 1), 16: (1, 0), 28: (1, 1)}
        prev = None
        for i, task in enumerate(tasks):
            if i in load_at:
                load_x(*load_at[i], nc.scalar)
            if i in planes_at:
                make_planes(*planes_at[i])
            c1 = conv_stage(*task)
            if prev is not None:
                mix_stage(*prev[0], prev[1])
            prev = (task, c1)
        mix_stage(*prev[0], prev[1])

    nc.compile()
    return nc


def _get_nc():
    if "nc" not in _NC_CACHE:
        _NC_CACHE["nc"] = _build()
    return _NC_CACHE["nc"]


def _prep_inputs(x, dictionary, lookup_coefficients, lookup_indices):
    x = np.asarray(x, dtype=np.float32)
    dic = np.asarray(dictionary, dtype=np.float32)
    coeff = np.asarray(lookup_coefficients, dtype=np.float32).reshape(O, -1)
    idx = np.asarray(lookup_indices).astype(np.int64).reshape(O, -1)

    wmat = np.zeros((O, D), np.float32)
    np.add.at(wmat, (np.arange(O)[:, None], idx), coeff)
    wm = np.ascontiguousarray(wmat.T)                     # [D, O]

    dt_ = dic.transpose(1, 0, 2, 3)                       # [cin, d, kh, kw]
    wa = np.zeros((128, 3 * D), np.float32)
    wb = np.zeros((128, 2 * D), np.float32)
    for kh in range(3):
        wa[0:64, kh * D:(kh + 1) * D] = dt_[:, :, kh, 0]
        wa[64:128, kh * D:(kh + 1) * D] = dt_[:, :, kh, 2]
    wb[0:64, 0:D] = dt_[:, :, 0, 1]
    wb[64:128, 0:D] = dt_[:, :, 1, 1]
    wb[0:64, D:2 * D] = dt_[:, :, 2, 1]                   # rows 64.. stay zero

    xpad = np.zeros((B, CIN, PH, PW), ml_dtypes.bfloat16)
    xpad[:, :, 1:H + 1, 1:W + 1] = x.astype(ml_dtypes.bfloat16)
    wc = np.zeros((128, 5 * D + O), np.float32)
    wc[:, 0:3 * D] = wa
    wc[:, 3 * D:5 * D] = wb
    wc[0:D, 5 * D:5 * D + O] = wm
    wc = wc.astype(ml_dtypes.bfloat16)

    in_maps = []
    for c in range(NCORES):
        xc = xpad[c * BPC:(c + 1) * BPC].transpose(1, 0, 2, 3).reshape(CIN, F)
        in_maps.append({
            "xp": np.ascontiguousarray(xc),
            "wc": wc,
        })
    return in_maps


def _run(in_maps, trace=False, **kw):
    nc = _get_nc()
    return run_bass_kernel_spmd(nc, in_maps, core_ids=list(range(NCORES)),
                                trace=trace, **kw)


def _unshuffle(raw):
    # staging order [BPC*NG, 128, u*G*N + t*N + r*W + w] -> [BPC, O, H, W]
    arr = np.asarray(raw, dtype=np.float32).reshape(BPC, NG, 128, 2, G, R, W)
    return arr.transpose(0, 3, 2, 1, 4, 5, 6).reshape(BPC, O, H, W)


def kernel(x, dictionary, lookup_coefficients, lookup_indices):
    in_maps = _prep_inputs(x, dictionary, lookup_coefficients, lookup_indices)
    res = _run(in_maps)
    outs = [_unshuffle(res.results[c]["out"]) for c in range(NCORES)]
    return np.concatenate(outs, axis=0)


# revision 41
# speedup vs baseline: 1.0674x; 1.0674x over previous
"""LCNN conv2d kernel for Trainium2 (8 NeuronCores, batch-sharded).

Math: out[b,o,h,w] = sum_d Wmat[o,d] * conv2d(x, dictionary)[b,d,h,w]
where Wmat is the scatter-add of lookup_coefficients into [O, D].

Device strategy (per core, 2 batches), all matmuls bf16 (PE reaches the
2.4 GHz pstate; f32r holds it near 1.2 GHz):
 - stage 1: conv with the D=100 dictionary in 5 accumulating matmuls per
   output tile. Two shifted copies of x pack two kernel taps into the
   128-partition contraction: XA pairs (kh,0)+(kh,1) via a +1 shift,
   XB pairs (0,2)+(1,2) via a +PW shift; only tap (2,2) runs alone.
 - stage 2: [O=256, D=100] channel-mix matmul on the conv result.
 - output staged and stored as bf16 (halves HBM store traffic); host
   upcasts to f32.
"""
import os
import sys

for _p in ("/opt/trn_rl_repo", "/root/.axon_site/_ro/trn_rl_repo"):
    if os.path.isdir(_p) and _p not in sys.path:
        sys.path.insert(0, _p)

import ml_dtypes
import numpy as np
from contextlib import ExitStack

from concourse import bacc, mybir, tile
from concourse.bass_utils import run_bass_kernel_spmd

# problem shapes (hardcoded per contract)
B, CIN, H, W = 16, 64, 96, 96
D, O = 100, 256
NCORES = 8
BPC = B // NCORES          # batches per core
PH, PW = H + 2, W + 2      # zero-padded spatial
F = BPC * PH * PW          # per-partition x extent
R = 4                      # output rows per matmul tile
NT = H // R                # h-tiles per batch
G = 4                      # h-tiles per output-DMA group
NG = NT // G
N = R * W                  # matmul free size (384)
f32 = mybir.dt.float32
bf16 = mybir.dt.bfloat16

_NC_CACHE = {}


def _build():
    nc = bacc.Bacc(None, target_bir_lowering=False, debug=False)
    # host pre-casts to bf16 so loads use the fast no-cast HWDGE path; x is
    # loaded once (2.4 MB) and the three shifted tap-planes are built with
    # SBUF->SBUF DMA (no HBM traffic).
    xp = nc.declare_dram_parameter("xp", [CIN, F], bf16, isOutput=False)
    # all weights in one buffer: one DMA descriptor (issue instructions
    # cost ~700ns each on the in-order queues)
    WTOT = 5 * D + O
    wc = nc.declare_dram_parameter("wc", [128, WTOT], bf16, isOutput=False)
    # output in staging-buffer order (one flat 2D HWDGE store per group;
    # 3D/strided APs would fall back to the ~155 GB/s software DGE); the
    # host un-shuffles for free.
    out = nc.declare_dram_parameter("out", [BPC * NG, 128, 2 * G * N], bf16,
                                    isOutput=True)

    # h-rows covered by each x sub-tile (two overlapping sub-tiles per
    # batch so every conv tile's 6-row window lives in exactly one).
    SOFF = NT // 2 * R         # second sub-tile starts at h=48
    SH = SOFF + 2              # 50 rows each (= PH - SOFF)

    with tile.TileContext(nc) as tc, ExitStack() as ctx:
        sb = ctx.enter_context(tc.tile_pool(name="sb", bufs=1))
        conv1p = ctx.enter_context(tc.tile_pool(name="conv1p", bufs=3))
        stgp = ctx.enter_context(tc.tile_pool(name="stgp", bufs=2))
        pcp = ctx.enter_context(tc.tile_pool(name="pcp", bufs=2, space="PSUM"))
        pop = ctx.enter_context(tc.tile_pool(name="pop", bufs=2, space="PSUM"))

        wc_s = sb.tile([128, WTOT], bf16)
        wa_s = wc_s[:, 0:3 * D]
        wb_s = wc_s[:, 3 * D:5 * D]
        wm_s = wc_s[0:D, 5 * D:5 * D + O]

        # x planes as 4 sub-tiles each (per batch, overlapping half
        # heights). XA = [x; x shifted +2] pairs taps (kh,0)+(kh,2);
        # XB = [x; x shifted +PW] pairs (0,1)+(1,1); only (2,1) runs
        # alone. All shifts are 4-byte aligned, so the planes are built by
        # fast-path DVE/Act copies from the single HBM x load — the DMA
        # engines (the scarcest resource: ~220ns/packet, ~235 GB/s
        # ceiling shared with the output store) move x only once.
        SL = SH * PW
        XAs = [[None, None] for _ in range(BPC)]
        XBs = [[None, None] for _ in range(BPC)]
        for b in range(BPC):
            for s in range(2):
                XAs[b][s] = sb.tile([128, SL], bf16, name=f"xa_{b}_{s}")
                XBs[b][s] = sb.tile([128, SL], bf16, name=f"xb_{b}_{s}")
                # finite filler for tails the zero-weighted or unread upper
                # rows cover (NaN would poison the PSUM)
                nc.gpsimd.memset(XAs[b][s][64:128, SL - 2:SL], 0.0)
                nc.gpsimd.memset(XBs[b][s][64:128, SL - PW:SL], 0.0)

        def load_x(b, s, eng, lo=0, hi=SL):
            XA = XAs[b][s]
            base = (b * PH + s * SOFF) * PW
            eng.dma_start(XA[0:CIN, lo:hi], xp[:, base + lo:base + hi])

        def make_planes(b, s, lo=0, hi=SL):
            # all on DVE: the Act engine runs these 64-partition copies 3x
            # slower and they'd block the evacuation stream
            XA, XB = XAs[b][s], XBs[b][s]
            nc.vector.tensor_copy(XA[64:128, lo:min(hi, SL - 2)],
                                  XA[0:CIN, lo + 2:min(hi + 2, SL)])
            nc.vector.tensor_copy(XB[0:CIN, lo:hi], XA[0:CIN, lo:hi])
            nc.vector.tensor_copy(XB[64:128, lo:min(hi, SL - PW)],
                                  XA[0:CIN, lo + PW:min(hi + PW, SL)])

        # upfront loads: weights + first sub-tile piece ONLY (the DMA
        # engines fair-share across every pending descriptor, so anything
        # else issued now delays the critical first-tile data). b0s1 and
        # batch 1 are issued from the paced scalar stream inside the loop.
        HSL = SL // 2
        nc.sync.dma_start(wc_s[:], wc[:])
        load_x(0, 0, nc.sync, 0, HSL + PW)
        make_planes(0, 0, 0, HSL)
        load_x(0, 0, nc.sync, HSL + PW, SL)
        make_planes(0, 0, HSL, SL)
        load_x(0, 1, nc.sync)

        def conv_stage(b, g, t):
            h0 = (g * G + t) * R
            s = 1 if h0 >= SOFF else 0
            hl = h0 - s * SOFF
            xav = XAs[b][s].rearrange("p (h w) -> p h w", h=SH, w=PW)
            xbv = XBs[b][s].rearrange("p (h w) -> p h w", h=SH, w=PW)
            pc = pcp.tile([D, N], f32, name="pc")
            # taps (kh,0)+(kh,2) for kh=0,1,2 via the +2 shift
            for kh in range(3):
                nc.tensor.matmul(
                    pc[:], wa_s[:, kh * D:(kh + 1) * D],
                    xav[:, hl + kh:hl + kh + R, 0:W],
                    start=(kh == 0), stop=False)
            # taps (0,1)+(1,1) via the +PW shift
            nc.tensor.matmul(
                pc[:], wb_s[:, 0:D],
                xbv[:, hl:hl + R, 1:1 + W],
                start=False, stop=False)
            # tap (2,1) alone: upper weight rows are zero, upper data rows
            # only need to be finite
            nc.tensor.matmul(
                pc[:], wb_s[:, D:2 * D],
                xbv[:, hl + 2:hl + 2 + R, 1:1 + W],
                start=False, stop=True)
            # PSUM conv evacuation on Act (DVE carries po1 + plane copies)
            c1 = conv1p.tile([D, N], bf16, name="c1")
            nc.scalar.copy(c1[:], pc[:])
            return c1

        stg_of = {}

        def mix_stage(b, g, t, c1):
            if t == 0:
                stg_of[(b, g)] = stgp.tile([128, 2 * G * N], bf16, name="stg")
            stg = stg_of[(b, g)]
            po0 = pop.tile([128, N], f32, name="po0")
            po1 = pop.tile([128, N], f32, name="po1")
            nc.tensor.matmul(po0[:], wm_s[:, 0:128], c1[:],
                             start=True, stop=True)
            nc.tensor.matmul(po1[:], wm_s[:, 128:256], c1[:],
                             start=True, stop=True)
            nc.scalar.copy(stg[:, t * N:(t + 1) * N], po0[:])
            nc.vector.tensor_copy(
                stg[:, G * N + t * N:G * N + (t + 1) * N], po1[:])
            if b == BPC - 1 and g == NG - 1 and t % 2 == 1:
                # last group: store per pair-of-tiles halves so the kernel
                # tail is one small store, not a whole-group one
                h0_, h1_ = (t - 1) * N, (t + 1) * N
                nc.sync.dma_start(out[b * NG + g][:, h0_:h1_],
                                  stg[:, h0_:h1_])
                nc.sync.dma_start(
                    out[b * NG + g][:, G * N + h0_:G * N + h1_],
                    stg[:, G * N + h0_:G * N + h1_])
            elif t == G - 1 and not (b == BPC - 1 and g == NG - 1):
                # one flat 2D store per group -> hardware DGE; issued from
                # sync, which is idle after the initial loads
                nc.sync.dma_start(out[b * NG + g], stg[:])

        # software pipeline: the mix matmuls of tile i-1 are issued after
        # the conv matmuls of tile i, so the PE never stalls on the c1
        # PSUM->SBUF evacuation latency.
        tasks = [(b, g, t)
                 for b in range(BPC) for g in range(NG) for t in range(G)]
        # staged issues, each placed so its inputs are long since ready and
        # it lands well before first use (b0s1 used at task 12, b1s0 at
        # 24, b1s1 at 36)
        load_at = {6: (1, 0), 18: (1, 1)}
        planes_at = {6: (0, 1), 16: (1, 0), 28: (1, 1)}
        prev = None
        for i, task in enumerate(tasks):
            if i in load_at:
                load_x(*load_at[i], nc.scalar)
            if i in planes_at:
                make_planes(*planes_at[i])
            c1 = conv_stage(*task)
            if prev is not None:
                mix_stage(*prev[0], prev[1])
            prev = (task, c1)
        mix_stage(*prev[0], prev[1])

    nc.compile()
    return nc


def _get_nc():
    if "nc" not in _NC_CACHE:
        _NC_CACHE["nc"] = _build()
    return _NC_CACHE["nc"]


def _prep_inputs(x, dictionary, lookup_coefficients, lookup_indices):
    x = np.asarray(x, dtype=np.float32)
    dic = np.asarray(dictionary, dtype=np.float32)
    coeff = np.asarray(lookup_coefficients, dtype=np.float32).reshape(O, -1)
    idx = np.asarray(lookup_indices).astype(np.int64).reshape(O, -1)

    wmat = np.zeros((O, D), np.float32)
    np.add.at(wmat, (np.arange(O)[:, None], idx), coeff)
    wm = np.ascontiguousarray(wmat.T)                     # [D, O]

    dt_ = dic.transpose(1, 0, 2, 3)                       # [cin, d, kh, kw]
    wa = np.zeros((128, 3 * D), np.float32)
    wb = np.zeros((128, 2 * D), np.float32)
    for kh in range(3):
        wa[0:64, kh * D:(kh + 1) * D] = dt_[:, :, kh, 0]
        wa[64:128, kh * D:(kh + 1) * D] = dt_[:, :, kh, 2]
    wb[0:64, 0:D] = dt_[:, :, 0, 1]
    wb[64:128, 0:D] = dt_[:, :, 1, 1]
    wb[0:64, D:2 * D] = dt_[:, :, 2, 1]                   # rows 64.. stay zero

    xpad = np.zeros((B, CIN, PH, PW), ml_dtypes.bfloat16)
    xpad[:, :, 1:H + 1, 1:W + 1] = x.astype(ml_dtypes.bfloat16)
    wc = np.zeros((128, 5 * D + O), np.float32)
    wc[:, 0:3 * D] = wa
    wc[:, 3 * D:5 * D] = wb
    wc[0:D, 5 * D:5 * D + O] = wm
    wc = wc.astype(ml_dtypes.bfloat16)

    in_maps = []
    for c in range(NCORES):
        xc = xpad[c * BPC:(c + 1) * BPC].transpose(1, 0, 2, 3).reshape(CIN, F)
        in_maps.append({
            "xp": np.ascontiguousarray(xc),
            "wc": wc,
        })
    return in_maps


def _run(in_maps, trace=False, **kw):
    nc = _get_nc()
    return run_bass_kernel_spmd(nc, in_maps, core_ids=list(range(NCORES)),
                                trace=trace, **kw)


def _unshuffle(raw):
    # staging order [BPC*NG, 128, u*G*N + t*N + r*W + w] -> [BPC, O, H, W]
    arr = np.asarray(raw, dtype=np.float32).reshape(BPC, NG, 128, 2, G, R, W)
    return arr.transpose(0, 3, 2, 1, 4, 5, 6).reshape(BPC, O, H, W)


def kernel(x, dictionary, lookup_coefficients, lookup_indices):
    in_maps = _prep_inputs(x, dictionary, lookup_coefficients, lookup_indices)
    res = _run(in_maps)
    outs = [_unshuffle(res.results[c]["out"]) for c in range(NCORES)]
    return np.concatenate(outs, axis=0)
